# revision 9
# baseline (speedup 1.0000x reference)
"""MoE layer (top-2 of 8 experts) on 8 TRN2 NeuronCores.

Strategy (single device launch):
  Host: gate logits (tiny 8192x1024x8 sgemm), softmax + top-2 +
      renormalized weights, and the expert-parallel sharding decision.
  Device (one SPMD launch, 8 cores): each core runs 2 "slots"; a slot
      is (expert e, half of H) and processes all tokens routed to e:
      partial_y = relu(x @ W1[e][:, half] + b1) @ W2[e][half, :] * w_tok.
      Half-experts are assigned to slots sorted-balanced (big halves to
      one slot group, small to the other) so per-core work is ~sum n_e/8
      instead of max_e n_e. bf16 matmuls, fp32 PSUM.
  Host: sum the two H-halves and scatter-add the two scaled expert
      contributions per token (+ w-weighted b2 correction).

Startup-critical path: the first slot's W1 arrives in chunk tiles (first
chunk 512KB, ahead of b1/wt on the scalar ring) and block 0's x arrives
in per-d-tile chunks on the sync ring, so the first matmul needs only
~640KB of DMA. The tail-less slot runs first so the program drains on a
128-token block whose two output DMAs go on different rings.
"""

import numpy as np
import ml_dtypes

import concourse.mybir as mybir
import concourse.tile as tile
from concourse import bacc
from concourse.bass_utils import run_bass_kernel_spmd

P = 128
N_CORES = 8
NS = 2          # slots (half-experts) per core
CB = 512        # token block
BF16 = mybir.dt.bfloat16
F32 = mybir.dt.float32
_bf16_np = ml_dtypes.bfloat16

_build_cache = {}


def _blocks(cap):
    """CB-sized token blocks with trailing 128-blocks."""
    out, pos = [], 0
    while pos < cap:
        cb = CB if cap - pos >= CB else P
        out.append((pos, cb))
        pos += cb
    return out


def _build_moe(D, HQ, O, caps):
    """Per-core program: NS slots, slot s = one (expert, H-half) over
    caps[s] padded routed tokens.

    Layer 1 keeps H on partitions (hT = W1-half.T-slices @ xT), layer 2
    puts tokens back on partitions (y = hT-slices.T @ W2-half). Weights
    arrive host-prearranged in SBUF layout; x/b1/wt host-prearranged.
    """
    DO, HO, OO = D // P, HQ // P, O // 512
    CT = sum(caps)
    nc = bacc.Bacc(None, target_bir_lowering=False)
    xT = nc.dram_tensor("xT", [D, CT], BF16, kind="ExternalInput")
    w1 = nc.dram_tensor("w1", [P, NS, DO, HQ], BF16, kind="ExternalInput")
    w2 = nc.dram_tensor("w2", [P, NS, HO, O], BF16, kind="ExternalInput")
    b1 = nc.dram_tensor("b1", [P, NS * HO], F32, kind="ExternalInput")
    wt = nc.dram_tensor("wt", [P, CT // P], F32, kind="ExternalInput")
    y = nc.dram_tensor("y", [CT, O], F32, kind="ExternalOutput")
    xT_r = xT.rearrange("(do p) c -> p do c", p=P)
    y_r = y.rearrange("(n p) o -> p n o", p=P)
    # first slot's W1 h-column chunks (first one small: block-0 matmuls
    # start after ~512KB) and W2 halves; later slots load whole
    W1CH = [(h0, 2 * P) for h0 in range(0, min(8 * P, HQ), 2 * P)] + [
        (h0, 4 * P) for h0 in range(8 * P, HQ, 4 * P)
    ]
    n_ybr = sum(1 for c in caps for _ in _blocks(c))  # total blocks
    with tile.TileContext(nc) as tc:
        with (
            tc.tile_pool(name="wp", bufs=1) as wp,
            tc.tile_pool(name="xp", bufs=2) as xp,
            tc.tile_pool(name="x0p", bufs=1) as x0p,
            tc.tile_pool(name="hp", bufs=2) as hp,
            tc.tile_pool(name="op", bufs=4) as op,
            tc.tile_pool(name="hps", bufs=3, space="PSUM") as hps,
            tc.tile_pool(name="yps", bufs=4, space="PSUM") as yps,
        ):
            # --- weight / const streams (scalar HWDGE ring, in order) ---
            w1f = []            # first slot W1: (h0, tile of [P, DO, nh])
            for k, (h0, nh) in enumerate(W1CH):
                t = wp.tile([P, DO, nh], BF16, tag=f"w1f{k}", name=f"w1f{k}")
                nc.scalar.dma_start(t[:], w1[:, 0, :, h0:h0 + nh])
                w1f.append((h0, nh, t))
                if k == 0:
                    # b1 is needed at the first relu (~13us); wt only at
                    # the first output scale (~40us)
                    b1_sb = wp.tile([P, NS * HO], F32, tag="b1", name="b1_sb")
                    nc.scalar.dma_start(b1_sb[:], b1[:])
                if k == len(W1CH) - 1:
                    wt_sb = wp.tile([P, CT // P], F32, tag="wt", name="wt_sb")
                    nc.scalar.dma_start(wt_sb[:], wt[:])
            w2f = []            # first slot W2 in two half tiles
            for k in range(2):
                t = wp.tile([P, HO // 2, O], BF16, tag=f"w2f{k}", name=f"w2f{k}")
                nc.scalar.dma_start(t[:], w2[:, 0, k * HO // 2:(k + 1) * HO // 2])
                w2f.append(t)
            w1r = [None]        # later slots: whole tiles
            w2r = [None]
            for s in range(1, NS):
                t1 = wp.tile([P, DO, HQ], BF16, tag=f"w1_{s}", name=f"w1_{s}")
                nc.scalar.dma_start(t1[:], w1[:, s])
                t2 = wp.tile([P, HO, O], BF16, tag=f"w2_{s}", name=f"w2_{s}")
                nc.scalar.dma_start(t2[:], w2[:, s])
                w1r.append(t1)
                w2r.append(t2)

            def w1_slice(s, di, hi):
                if s > 0:
                    return w1r[s][:, di, hi * P:(hi + 1) * P]
                for (h0, nh, t) in w1f:
                    if h0 <= hi * P < h0 + nh:
                        return t[:, di, hi * P - h0:(hi + 1) * P - h0]
                raise AssertionError

            def w2_slice(s, hi, ot):
                if s > 0:
                    return w2r[s][:, hi, ot * 512:(ot + 1) * 512]
                return w2f[hi // (HO // 2)][:, hi % (HO // 2), ot * 512:(ot + 1) * 512]

            # --- main loop ---
            off = 0
            blk_idx = 0
            for s in range(NS):
                for (n0, cb) in _blocks(caps[s]):
                    g0 = off + n0
                    first = blk_idx == 0
                    last = blk_idx == n_ybr - 1
                    if first:
                        # per-d-tile x chunks: first matmul waits on 1/8
                        xds = []
                        for di in range(DO):
                            xt = x0p.tile([P, CB], BF16, tag=f"x0d{di}",
                                          name=f"x0d{di}")[:, :cb]
                            nc.sync.dma_start(xt[:], xT_r[:, di, g0:g0 + cb])
                            xds.append(xt)
                        x_of = lambda di: xds[di]
                    else:
                        x_sb = xp.tile([P, DO, CB], BF16, tag="x",
                                       name="x_sb")[:, :, :cb]
                        nc.sync.dma_start(x_sb[:], xT_r[:, :, g0:g0 + cb])
                        x_of = lambda di: x_sb[:, di]
                    hT = hp.tile([P, HO, CB], BF16, tag="h", name="hT")[:, :, :cb]
                    for hi in range(HO):
                        ph = hps.tile([P, CB], F32, tag="ph", name="ph")[:, :cb]
                        for di in range(DO):
                            nc.tensor.matmul(
                                ph[:],
                                w1_slice(s, di, hi),
                                x_of(di),
                                start=(di == 0),
                                stop=(di == DO - 1),
                            )
                        nc.scalar.activation(
                            hT[:, hi], ph[:],
                            mybir.ActivationFunctionType.Relu,
                            bias=b1_sb[:, s * HO + hi:s * HO + hi + 1],
                        )
                    for ct in range(cb // P):
                        # hi outer / ot inner: both ot matmuls share the
                        # same stationary hT slice
                        yts = [yps.tile([P, 512], F32, tag="yp", name=f"yp{ot}")
                               for ot in range(OO)]
                        for hi in range(HO):
                            for ot in range(OO):
                                nc.tensor.matmul(
                                    yts[ot][:],
                                    hT[:, hi, ct * P:(ct + 1) * P],
                                    w2_slice(s, hi, ot),
                                    start=(hi == 0),
                                    stop=(hi == HO - 1),
                                )
                        ncol = g0 // P + ct
                        for ot in range(OO):
                            o_sb = op.tile([P, 512], F32, tag="o")
                            nc.vector.tensor_scalar_mul(
                                o_sb[:], yts[ot][:], wt_sb[:, ncol:ncol + 1]
                            )
                            # split the drain of the very last block
                            eng = nc.scalar if (last and ot == 1) else nc.sync
                            eng.dma_start(
                                y_r[:, ncol, ot * 512:(ot + 1) * 512], o_sb[:]
                            )
                    blk_idx += 1
                off += caps[s]
    nc.finalize()
    return nc


def _pad128(n):
    return max(P, ((n + P - 1) // P) * P)


def kernel(x, W1, b1, W2, b2, gate_w, gate_b):
    x = np.ascontiguousarray(x, dtype=np.float32)
    W1 = np.asarray(W1, dtype=np.float32)
    b1 = np.asarray(b1, dtype=np.float32)
    W2 = np.asarray(W2, dtype=np.float32)
    b2 = np.asarray(b2, dtype=np.float32)
    gate_w = np.ascontiguousarray(gate_w, dtype=np.float32)
    gate_b = np.asarray(gate_b, dtype=np.float32)

    B, D = x.shape
    E, _, H = W1.shape
    O = W2.shape[2]
    HQ = H // NS
    HO = HQ // P
    assert E == N_CORES and D % P == 0 and H % (NS * P) == 0

    # ---- Host: gating + top-2 routing (the sharding decision) ----
    lg = x.astype(np.float64) @ gate_w.astype(np.float64) + gate_b
    lg -= lg.max(axis=1, keepdims=True)
    probs = np.exp(lg)
    probs /= probs.sum(axis=1, keepdims=True)
    order = np.argsort(-probs, axis=1, kind="stable")[:, :2]
    p_top = np.take_along_axis(probs, order, axis=1)
    w_top = (p_top / p_top.sum(axis=1, keepdims=True)).astype(np.float32)

    idx_e, wt_e = [], []
    for e in range(E):
        m0 = order[:, 0] == e
        m1 = order[:, 1] == e
        sel = m0 | m1
        idx = np.nonzero(sel)[0]
        w = np.where(m0[sel], w_top[sel, 0], w_top[sel, 1]).astype(np.float32)
        idx_e.append(idx)
        wt_e.append(w)

    # ---- Balanced slot assignment: NS half-experts per core ----
    units = sorted(
        [(len(idx_e[e]), e, q) for e in range(E) for q in range(NS)],
        key=lambda t: (-t[0], t[1], t[2]),
    )
    groups = [units[p * N_CORES:(p + 1) * N_CORES] for p in range(NS)]
    caps = [_pad128(max(u[0] for u in g)) for g in groups]
    # process tail-less slots first so the program drains on a 128-block
    proc = sorted(range(NS), key=lambda s: (caps[s] % CB != 0, -caps[s]))
    groups = [groups[s] for s in proc]
    caps = tuple(caps[s] for s in proc)
    CT = sum(caps)

    key = ("moe", D, HQ, O, caps)
    if key not in _build_cache:
        _build_cache[key] = _build_moe(D, HQ, O, caps)
    nc = _build_cache[key]

    # ---- Build per-core inputs ----
    x_bf = x.astype(_bf16_np)
    xTe = {e: np.ascontiguousarray(x_bf[idx_e[e]].T) for e in range(E)}
    W1_bf = W1.astype(_bf16_np)
    W2_bf = W2.astype(_bf16_np)
    in_maps = []
    for c in range(N_CORES):
        slots = [groups[p][c] for p in range(NS)]
        xT = np.zeros((D, CT), dtype=_bf16_np)
        w1h = np.empty((P, NS, D // P, HQ), dtype=_bf16_np)
        w2h = np.empty((P, NS, HO, O), dtype=_bf16_np)
        b1h = np.zeros((P, NS * HO), dtype=np.float32)
        wth = np.zeros((P, CT // P), dtype=np.float32)
        off = 0
        for s, (n_u, e, q) in enumerate(slots):
            hsl = slice(q * HQ, (q + 1) * HQ)
            xT[:, off:off + n_u] = xTe[e]
            w1h[:, s] = W1_bf[e][:, hsl].reshape(D // P, P, HQ).transpose(1, 0, 2)
            w2h[:, s] = W2_bf[e][hsl].reshape(HO, P, O).transpose(1, 0, 2)
            b1h[:, s * HO:(s + 1) * HO] = b1[e][hsl].reshape(HO, P).T
            wpad = np.zeros(caps[s], dtype=np.float32)
            wpad[:n_u] = wt_e[e]
            wth[:, off // P:(off + caps[s]) // P] = wpad.reshape(-1, P).T
            off += caps[s]
        in_maps.append({
            "xT": xT,
            "w1": np.ascontiguousarray(w1h),
            "w2": np.ascontiguousarray(w2h),
            "b1": b1h,
            "wt": wth,
        })

    res = run_bass_kernel_spmd(nc, in_maps, core_ids=list(range(N_CORES)))

    # ---- Host: combine H-halves / experts, add gated b2 ----
    out = np.zeros((B, O), dtype=np.float32)
    for c in range(N_CORES):
        yc = res.results[c]["y"]
        off = 0
        for s in range(NS):
            n_u, e, q = groups[s][c]
            if n_u:
                out[idx_e[e]] += yc[off:off + n_u]
            off += caps[s]
    if np.any(b2):
        out += w_top[:, 0, None] * b2[order[:, 0]]
        out += w_top[:, 1, None] * b2[order[:, 1]]
    return out
